# revision 4
# baseline (speedup 1.0000x reference)
"""Dual attention (DANet-style spatial + channel attention) on 8 Trainium2
NeuronCores.

Sharding: data-parallel over batch B=4, each batch's output positions split in
half across 2 cores -> 8 identical single-core programs, no collectives. The
host permutes each core's x so its OWN m-half occupies columns [0, M).

Per-core math (x: [512, 4096] f16, m-chunk: 2048 positions):
  spatial:  q=Wq@x[:, :M]+bq; k=Wk@x+bk; E^T[n,m]=k[:,n].q[:,m]
            p_t[n,m] = exp(E - 45)  (bf16, ACT bias)
            S1[m] = sum_n p_t  via tiny PE ones-matmuls (f32 PSUM)
            sbcast = broadcast(1/S1) via 4 tiny PE transposes + outer product
            P~ = e4m3(p_t * sbcast)  -- normalized softmax weights in fp8
            U^T[m,c] = sum_n P~ vT8  via fp8 DoubleRow pairs (2 n-tiles/pass)
            out = gamma_s * U + R    (no 1/Z: P~ is already normalized)
  channel:  pT[n,d]=(Wd@x+bd)^T; e=pT^T@pT; c_attn=softmax(rowmax(e)-e)
            c2=gamma_c*(c_attn@p)[:, :M]+p[:, :M]
            R^T[m,c]=Wu@c2+bu+x[:, :M]^T  (residual DMA'd into r_sb, in-place
            add; bu via appended ones-row)
  out^T[m,c] f16 -> DRAM [M, C], host transposes.

Perf structure (vs the 245us bf16-U version):
  - U matmuls in fp8e4 DoubleRow: the softmax weights are normalized to
    (0, 1] by construction (exact column-sum shift), so they fit e4m3 with
    no range machinery; ~1.6x on the dominant PE cost.
  - the S-accumulation chain moved off the DVE onto tiny PE ones-matmuls
    (one [128,1] matmul per (tile, m-subtile), one PSUM group per chunk).
  - the DVE's new cost is one [128,1024] multiply->fp8 per tile pair
    (issued one pair AHEAD of the consuming U group so the PE never waits).
  - vT stored e4m3 (DR rhs), r_sb f16 (residual DMA'd straight into it),
    output f16 (halves out-DMA).
"""
import sys

sys.path.insert(0, '/opt/trn_rl_repo')

import numpy as np

import concourse.bass as bass
import concourse.tile as tile
from concourse import bacc, bass_utils, mybir
from concourse.masks import make_identity

B, C, WIDTH, HEIGHT = 4, 512, 64, 64
N = WIDTH * HEIGHT      # 4096 spatial positions
DK = 64                 # attention inner dim (and channel-attn dim)
NCORES = 8
M = N // 2              # 2048 output positions per core
P = 128
KC = C // P             # 4 input-channel chunks
NT = N // P             # 32 key-position tiles
NP = NT // 2            # 16 key-tile PAIRS (fp8 DoubleRow granularity)
FREE = 512              # matmul moving free dim (one PSUM bank of fp32)
MCH = M // FREE         # 4 m-chunks per core
MS = FREE // P          # 4 m-subtiles (128 rows) per chunk
MT = M // P             # 16 m-subtiles total

F32 = mybir.dt.float32
F16 = mybir.dt.float16
BF16 = mybir.dt.bfloat16
FP8 = mybir.dt.float8e4
AX = mybir.AxisListType
ALU = mybir.AluOpType
ACTF = mybir.ActivationFunctionType
DR = mybir.MatmulPerfMode.DoubleRow

EXP_BIAS = -45.0        # exp(E + EXP_BIAS): keeps bf16/f32 mid-chain in range

# byte layout of the packed-constants image (per partition)
OFF_WQ, OFF_WK = 0, 1024     # [128, kc, 128] f16: [Wq|Wq], [Wk|Wk] doubled
OFF_WD = 2048                # [128, kc, 64] f16
OFF_BQ, OFF_BK, OFF_BD, OFF_GC = 2560, 2564, 2568, 2572
OFF_GS = 2576                # [128, 1] f32, replicated on all partitions
OFF_P2 = 2592                # consts DMA split: everything below lands first
OFF_WUB = 2592               # [65, 512] f16: rows 0-63 Wu^T, row 64 = bu
OFF_BDB = 3616               # [128, 64] f32, bd row replicated on all partitions
OFF_BVB = 3872               # [128, 512] f32, bv row replicated on all partitions
PKB = 5920


def _build_program(tc, io):
    nc = tc.nc
    x_d, xmT_d, out_d = io['x'], io['xmT'], io['out']

    const_cm = tc.tile_pool(name='const', bufs=1)
    const = const_cm.__enter__()

    # ---- persistent SBUF tensors ----
    pk_sb = const.tile([P, PKB], mybir.dt.uint8)
    nc.sync.dma_start(pk_sb[:, 0:OFF_P2], io['consts'][:, 0:OFF_P2])
    wq_sb = pk_sb[:, OFF_WQ:OFF_WQ + 1024].bitcast(F16).rearrange(
        "p (kc d) -> p kc d", kc=KC)
    wk_sb = pk_sb[:, OFF_WK:OFF_WK + 1024].bitcast(F16).rearrange(
        "p (kc d) -> p kc d", kc=KC)
    wd_sb = pk_sb[:, OFF_WD:OFF_WD + 512].bitcast(F16).rearrange(
        "p (kc d) -> p kc d", kc=KC)
    wub_sb = pk_sb[0:DK + 1, OFF_WUB:OFF_WUB + 1024].bitcast(F16)
    bq_sb = pk_sb[:, OFF_BQ:OFF_BQ + 4].bitcast(F32)
    bk_sb = pk_sb[:, OFF_BK:OFF_BK + 4].bitcast(F32)
    bd_sb = pk_sb[0:DK, OFF_BD:OFF_BD + 4].bitcast(F32)
    gc_sb = pk_sb[0:DK, OFF_GC:OFF_GC + 4].bitcast(F32)
    gs_sb = pk_sb[:, OFF_GS:OFF_GS + 4].bitcast(F32)
    bdb_sb = pk_sb[:, OFF_BDB:OFF_BDB + 256].bitcast(F32)
    bvb_sb = pk_sb[:, OFF_BVB:OFF_BVB + 2048].bitcast(F32)

    ones_colb = const.tile([P, 1], BF16)    # rhs for the tiny S1 matmuls
    nc.vector.memset(ones_colb[:], 1.0)
    ones_rowb = const.tile([1, P], BF16)    # lhsT for the sbcast outer product
    nc.vector.memset(ones_rowb[:], 1.0)
    bias45 = const.tile([P, 1], F32)        # exp bias
    nc.vector.memset(bias45[:], EXP_BIAS)
    ident16 = const.tile([DK, DK], F16)     # for the tiny c_attn transpose
    make_identity(nc, ident16[:])
    identb = const.tile([P, P], BF16)       # for the 1/S1 row transposes
    make_identity(nc, identb[:])

    k_sb = const.tile([P, N], F16)   # keys [d, n], rows 64-127 = copy
    q_sb = const.tile([P, M], F16)   # queries,   rows 64-127 = copy
    pc_sb = const.tile([DK, M], F16)       # channel proj on the m-slice
    c2b_sb = const.tile([DK + 1, M], F16)  # c2 rows 0-63, row 64 = ones
    pT_sb = const.tile([P, NT, DK], F16)   # channel proj transposed [n, nt, d]
    vT8_sb = const.tile([P, NT, C], FP8)   # values transposed e4m3, [n, nt, c]
    catT_sb = const.tile([DK, DK], F16)    # c_attn^T for the c2 matmuls
    r_sb = const.tile([P, MT, C], F16)     # R^T = channel-out + residual

    nc.vector.memset(c2b_sb[DK:DK + 1, :], 1.0)

    wv8_sb = const.tile([P, KC, C], FP8)   # fp8 Wv^T for DR

    out_r = out_d.rearrange("(mt p) c -> p mt c", p=P)

    # ---- pools ----
    # PSUM budget (8 banks): upool 4 + epool 2 + s1 1 + aux 1.
    # During the fused phase upool is not yet entered: epool 2 + s1 1 +
    # auxf 2 (pv/pt rotation) + ps0(3, conv scope) stay within 8.
    s1p_cm = tc.tile_pool(name='s1p', bufs=1, space='PSUM')
    s1p = s1p_cm.__enter__()
    epool_cm = tc.tile_pool(name='epool', bufs=2, space='PSUM')
    aux_cm = tc.tile_pool(name='aux', bufs=1, space='PSUM')
    upool_cm = tc.tile_pool(name='upool', bufs=MS, space='PSUM')
    upool = epool = aux = None

    ptp_cm = tc.tile_pool(name='pt', bufs=18)    # bf16 exp pairs [P, 2, FREE]
    ptp = ptp_cm.__enter__()
    p8p_cm = tc.tile_pool(name='p8', bufs=4)     # fp8 scaled pairs
    p8p = p8p_cm.__enter__()
    ssb_cm = tc.tile_pool(name='ssb', bufs=4)    # srh / srow / sbcast
    ssb = ssb_cm.__enter__()
    otp_cm = tc.tile_pool(name='ot', bufs=4)     # f16 epilogue tiles
    otp = otp_cm.__enter__()

    pair_tl = {}   # (mc, pj) -> bf16 exp pair tile
    s1_ps = {}     # mc -> [128, MS] f32 PSUM accumulation tile
    sbc = {}       # mc -> [P, 2, FREE] bf16 broadcast of 1/S1
    u_ps = {}      # mc -> list of MS U PSUM tiles

    def emit_e(mc, nt, epool):
        # nt parity picks the PE row-group: even tiles compute on array rows
        # 0-63, odd on 64-127 (k/q carry identical copies on partitions
        # 64-127), so adjacent E matmuls can overlap in the array.
        msl = slice(mc * FREE, (mc + 1) * FREE)
        nsl = slice(nt * P, (nt + 1) * P)
        h = (nt & 1) * DK
        e_t = epool.tile([P, FREE], F32, tag='et', name=f'et{mc}_{nt}')
        nc.tensor.matmul(e_t[:], lhsT=k_sb[h:h + DK, nsl],
                         rhs=q_sb[h:h + DK, msl],
                         start=True, stop=True, tile_position=(h, 0))
        pj = nt // 2
        if nt % 2 == 0:
            pair = ptp.tile([P, 2, FREE], BF16, tag='p', name=f'p{mc}_{pj}')
            pair_tl[(mc, pj)] = pair
        pair = pair_tl[(mc, pj)]
        nc.scalar.activation(pair[:, nt % 2], e_t[:], ACTF.Exp,
                             bias=bias45[:])

    def emit_s1(mc, nt):
        # S1[m] += column-sums of the exp tile: 4 tiny ones-matmuls into a
        # single per-chunk PSUM accumulation group (one start, one stop).
        if mc not in s1_ps:
            s1_ps[mc] = s1p.tile([P, MS], F32, tag='s1', name=f's1_{mc}')
        pair = pair_tl[(mc, nt // 2)]
        for ms in range(MS):
            nc.tensor.matmul(s1_ps[mc][:, ms:ms + 1],
                             lhsT=pair[:, nt % 2, ms * P:(ms + 1) * P],
                             rhs=ones_colb[:],
                             start=(nt == 0 and ms == 0),
                             stop=(nt == NT - 1 and ms == MS - 1))

    def emit_boundary(mc, aux):
        # After S1(mc) stops: 1/S1 -> bf16 row -> broadcast to [P, 2, FREE].
        srh = ssb.tile([P, MS], BF16, tag='srh', name=f'srh{mc}')
        with nc.allow_low_precision(reason='1/S1 scale: bf16 is plenty'):
            nc.vector.reciprocal(srh[:], s1_ps[mc][:])
        srow = ssb.tile([1, FREE], BF16, tag='srow', name=f'srow{mc}')
        for ms in range(MS):
            ttp = aux.tile([1, P], BF16, tag='aux', name=f'tr{mc}_{ms}')
            nc.tensor.transpose(ttp[:], srh[:, ms:ms + 1], identb[:])
            nc.vector.tensor_copy(srow[0:1, ms * P:(ms + 1) * P], ttp[:])
        sb_ps = aux.tile([P, FREE], F32, tag='aux', name=f'sbp{mc}')
        nc.tensor.matmul(sb_ps[:], lhsT=ones_rowb[:], rhs=srow[:],
                         start=True, stop=True)
        sbcast = ssb.tile([P, 2, FREE], BF16, tag='sbc', name=f'sbc{mc}')
        nc.vector.tensor_copy(sbcast[:, 0], sb_ps[:])
        nc.vector.tensor_copy(sbcast[:, 1], sb_ps[:])
        sbc[mc] = sbcast

    def emit_scale(mc, pj):
        # P~ pair = e4m3(p_t * 1/S1[m]) — one [128, 1024] DVE multiply.
        pair = pair_tl.pop((mc, pj))
        p8 = p8p.tile([P, 2, FREE], FP8, tag='p8', name=f'p8_{mc}_{pj}')
        nc.vector.tensor_mul(p8[:].rearrange("p a b -> p (a b)"),
                             in0=pair[:].rearrange("p a b -> p (a b)"),
                             in1=sbc[mc][:].rearrange("p a b -> p (a b)"))
        return p8

    def emit_u(mc, pj, p8):
        # 4 fp8 DoubleRow matmuls: two n-tiles per pass.
        for ms in range(MS):
            nc.tensor.matmul(u_ps[mc][ms][:],
                             lhsT=p8[:, :, ms * P:(ms + 1) * P],
                             rhs=vT8_sb[:, 2 * pj:2 * pj + 2],
                             start=(pj == 0), stop=(pj == NP - 1),
                             perf_mode=DR)

    def emit_combine(mc):
        # out = gamma_s * U + R, straight from PSUM (no Z: P~ is normalized).
        for ms in range(MS):
            o2 = otp.tile([P, FREE], F16, tag='o', name=f'o{mc}_{ms}')
            nc.vector.scalar_tensor_tensor(
                out=o2[:], in0=u_ps[mc][ms][:], scalar=gs_sb[:],
                in1=r_sb[:, mc * MS + ms], op0=ALU.mult, op1=ALU.add)
            nc.sync.dma_start(out_r[:, mc * MS + ms], o2[:])

    def channel_step(step, aux):
        # c2 = gamma_c * (c_attn @ p)[:, :M] + pc  (4 matmuls), then
        # R^T[mt] = (c2 | ones)^T @ (Wu^T | bu) + xmT  (16 matmuls, in-place
        # add into r_sb which already holds the residual).
        if step < MCH:
            j = step
            sl = slice(j * FREE, (j + 1) * FREE)
            co_ps = aux.tile([DK, FREE], F32, tag='aux', name=f'co{j}')
            nc.tensor.matmul(co_ps[:], lhsT=catT_sb[:], rhs=pc_sb[:, sl],
                             start=True, stop=True)
            nc.vector.scalar_tensor_tensor(
                out=c2b_sb[0:DK, sl], in0=co_ps[:], scalar=gc_sb[:],
                in1=pc_sb[:, sl], op0=ALU.mult, op1=ALU.add)
        else:
            mt = step - MCH
            rw_ps = aux.tile([P, C], F32, tag='aux', name=f'rw{mt}')
            nc.tensor.matmul(rw_ps[:], lhsT=c2b_sb[:, mt * P:(mt + 1) * P],
                             rhs=wub_sb[:], start=True, stop=True)
            nc.vector.tensor_add(r_sb[:, mt], in0=rw_ps[:], in1=r_sb[:, mt])

    # ================= phase 1 + fused chunk 0 =================
    with tc.tile_pool(name='xp', bufs=1) as xp:
        x_sb = xp.tile([P, KC, N], F16)
        x8_sb = xp.tile([P, KC, N], FP8)
        x_r = x_d.rearrange("(kc p) n -> p kc n", p=P)
        x8_r = io['x8'].rearrange("(kc p) n -> p kc n", p=P)
        for kc in range(KC):   # chunk 0 lands per-kc: conv 0 starts sooner
            nc.sync.dma_start(x_sb[:, kc, 0:FREE], x_r[:, kc, 0:FREE])
        nc.sync.dma_start(pk_sb[:, OFF_P2:PKB], io['consts'][:, OFF_P2:PKB])
        nc.sync.dma_start(x_sb[:, :, FREE:2 * FREE], x_r[:, :, FREE:2 * FREE])
        nc.sync.dma_start(x_sb[:, :, 2 * FREE:4 * FREE],
                          x_r[:, :, 2 * FREE:4 * FREE])
        nc.sync.dma_start(x_sb[:, :, 4 * FREE:6 * FREE],
                          x_r[:, :, 4 * FREE:6 * FREE])
        nc.sync.dma_start(x_sb[:, :, 6 * FREE:N], x_r[:, :, 6 * FREE:N])
        nc.sync.dma_start(wv8_sb[:],
                          io['wv8'].rearrange("(kc p) c -> p kc c", p=P))
        nc.sync.dma_start(x8_sb[:, :, 0:N // 2], x8_r[:, :, 0:N // 2])
        nc.sync.dma_start(x8_sb[:, :, N // 2:N], x8_r[:, :, N // 2:N])
        # residual lands straight in r_sb; channel tail adds in place
        nc.sync.dma_start(r_sb[:],
                          xmT_d.rearrange("(mt p) c -> p mt c", p=P))

        # conv projections, consuming x chunks as they land
        with tc.tile_pool(name='ps0', bufs=3, space='PSUM') as ps0:
            for j in range(8):
                sl = slice(j * FREE, (j + 1) * FREE)
                if j < MCH:
                    pq = ps0.tile([P, FREE], F32, tag='pq', name=f'pq{j}')
                    for kc in range(KC):
                        nc.tensor.matmul(pq[:], lhsT=wq_sb[:, kc],
                                         rhs=x_sb[:, kc, sl],
                                         start=(kc == 0), stop=(kc == KC - 1))
                    nc.scalar.activation(q_sb[:, sl], pq[:], ACTF.Identity,
                                         bias=bq_sb[:])
                    ppc = ps0.tile([DK, FREE], F32, tag='pq', name=f'ppc{j}')
                    for kc in range(KC):
                        nc.tensor.matmul(ppc[:], lhsT=wd_sb[:, kc],
                                         rhs=x_sb[:, kc, sl],
                                         start=(kc == 0), stop=(kc == KC - 1))
                    nc.scalar.activation(pc_sb[:, sl], ppc[:], ACTF.Identity,
                                         bias=bd_sb[:])
                pk = ps0.tile([P, FREE], F32, tag='pq', name=f'pk{j}')
                for kc in range(KC):
                    nc.tensor.matmul(pk[:], lhsT=wk_sb[:, kc],
                                     rhs=x_sb[:, kc, sl],
                                     start=(kc == 0), stop=(kc == KC - 1))
                nc.scalar.activation(k_sb[:, sl], pk[:], ACTF.Identity,
                                     bias=bk_sb[:])
        epool = epool_cm.__enter__()

        # fused chunk-0 loop: produce vT8/pT for tile nt, plus chunk 0's
        # E/exp/S1. No U here (U(0) waits for S1(0) -> runs next period).
        with tc.tile_pool(name='auxf', bufs=2, space='PSUM') as auxf:
            emit_e(0, 0, epool)
            emit_e(0, 1, epool)
            for nt in range(NT):
                nsl = slice(nt * P, (nt + 1) * P)
                pv = auxf.tile([P, C], F32, tag='aux', name=f'pv{nt}')
                for kcp in range(KC // 2):
                    nc.tensor.matmul(
                        pv[:], lhsT=x8_sb[:, 2 * kcp:2 * kcp + 2, nsl],
                        rhs=wv8_sb[:, 2 * kcp:2 * kcp + 2],
                        start=(kcp == 0), stop=(kcp == KC // 2 - 1),
                        perf_mode=DR)
                nc.vector.tensor_add(vT8_sb[:, nt], in0=pv[:], in1=bvb_sb[:])

                pt_ps = auxf.tile([P, DK], F32, tag='aux', name=f'ptp{nt}')
                for kc in range(KC):
                    nc.tensor.matmul(pt_ps[:], lhsT=x_sb[:, kc, nsl],
                                     rhs=wd_sb[:, kc],
                                     start=(kc == 0), stop=(kc == KC - 1))
                nc.vector.tensor_add(pT_sb[:, nt], in0=pt_ps[:], in1=bdb_sb[:])

                if nt % 2 == 1 and nt + 2 < NT:
                    emit_e(0, nt + 1, epool)
                    emit_e(0, nt + 2, epool)
                if nt >= 2:
                    # S1 lags the exp by 2 tiles so the in-order PE queue
                    # never blocks on a pending ACT exp semaphore
                    emit_s1(0, nt - 2)
            emit_s1(0, NT - 2)
            emit_s1(0, NT - 1)

    aux = aux_cm.__enter__()
    upool = upool_cm.__enter__()

    # ================= chunks: U(mc-1) + E/exp/S1(mc) =================
    for mc in range(1, MCH + 1):
        last = mc == MCH
        emit_boundary(mc - 1, aux)
        p8_next = emit_scale(mc - 1, 0)   # primed one pair ahead of U
        u_ps[mc - 1] = [upool.tile([P, FREE], F32, tag='u',
                                   name=f'u{mc - 1}_{i}') for i in range(MS)]
        if not last:
            emit_e(mc, 0, epool)
            emit_e(mc, 1, epool)
        for pj in range(NP):
            nt = 2 * pj
            if not last and pj >= 1:
                # S1 lags its exps by a full pair: no PE-queue stall
                emit_s1(mc, nt - 2)
                emit_s1(mc, nt - 1)
            p8 = p8_next
            if pj + 1 < NP:
                p8_next = emit_scale(mc - 1, pj + 1)
            emit_u(mc - 1, pj, p8)
            if not last and nt + 3 < NT:
                # E pair pj+1 issues after U(pj): exp(pair pj) has drained
                # by then, so the epool buffer rotation never blocks the PE
                emit_e(mc, nt + 2, epool)
                emit_e(mc, nt + 3, epool)
            if mc == 1:
                # channel attention interleave: gram over pT, softmax,
                # c_attn transpose, then c2 / R^T steps two per pair
                if pj < 4:
                    if pj == 0:
                        e_ps = aux.tile([DK, DK], F32, tag='aux',
                                        name='gram')
                    for g in range(8):
                        gi = pj * 8 + g
                        nc.tensor.matmul(e_ps[:], lhsT=pT_sb[:, gi],
                                         rhs=pT_sb[:, gi],
                                         start=(gi == 0),
                                         stop=(gi == NT - 1))
                if pj == 4:
                    # c_attn = softmax(rowmax(e) - e) == exp(rowmin - e)/sum
                    e_sb = const.tile([DK, DK], F32)
                    nc.vector.tensor_copy(e_sb[:], e_ps[:])
                    mn_sb = const.tile([DK, 1], F32)
                    nc.vector.tensor_reduce(mn_sb[:], e_sb[:], axis=AX.X,
                                            op=ALU.min)
                    h_sb = const.tile([DK, DK], F32)
                    nc.scalar.activation(h_sb[:], e_sb[:], ACTF.Exp,
                                         bias=mn_sb[:], scale=-1.0)
                    zc_sb = const.tile([DK, 1], F32)
                    nc.vector.tensor_reduce(zc_sb[:], h_sb[:], axis=AX.X,
                                            op=ALU.add)
                    nc.vector.reciprocal(zc_sb[:], zc_sb[:])
                    cat16_sb = const.tile([DK, DK], F16)
                    nc.vector.tensor_scalar_mul(cat16_sb[:], in0=h_sb[:],
                                                scalar1=zc_sb[:])
                if pj == 5:
                    catT_ps = aux.tile([DK, DK], F16, tag='aux',
                                       name='catT_ps')
                    nc.tensor.transpose(catT_ps[:], cat16_sb[:],
                                        ident16[:])
                    nc.vector.tensor_copy(catT_sb[:], catT_ps[:])
                if 6 <= pj:
                    channel_step(2 * (pj - 6), aux)
                    channel_step(2 * (pj - 6) + 1, aux)
        if not last:
            emit_s1(mc, NT - 2)
            emit_s1(mc, NT - 1)
        emit_combine(mc - 1)

    upool_cm.__exit__(None, None, None)
    aux_cm.__exit__(None, None, None)
    epool_cm.__exit__(None, None, None)
    otp_cm.__exit__(None, None, None)
    ssb_cm.__exit__(None, None, None)
    p8p_cm.__exit__(None, None, None)
    ptp_cm.__exit__(None, None, None)
    s1p_cm.__exit__(None, None, None)
    const_cm.__exit__(None, None, None)


_CACHE = {}


def _get_compiled():
    if 'nc' in _CACHE:
        return _CACHE['nc']
    nc = bacc.Bacc("TRN2", num_devices=NCORES)
    io = {
        'x': nc.dram_tensor('x', [C, N], F16, kind='ExternalInput').ap(),
        'x8': nc.dram_tensor('x8', [C, N], FP8, kind='ExternalInput').ap(),
        'wv8': nc.dram_tensor('wv8', [C, C], FP8, kind='ExternalInput').ap(),
        'xmT': nc.dram_tensor('xmT', [M, C], F16, kind='ExternalInput').ap(),
        'consts': nc.dram_tensor('consts', [P, PKB], mybir.dt.uint8,
                                 kind='ExternalInput').ap(),
        'out': nc.dram_tensor('out', [M, C], F16, kind='ExternalOutput').ap(),
    }
    with tile.TileContext(nc) as tc:
        _build_program(tc, io)
    nc.compile()
    _CACHE['nc'] = nc
    return nc


def make_in_maps(x, Wq, bq, Wk, bk, Wv, bv, gamma_s, Wd, bd, Wu, bu, gamma_c):
    """Build the 8 per-core input dicts from the full problem inputs."""
    f32 = lambda a: np.ascontiguousarray(np.asarray(a, dtype=np.float32))
    f16 = lambda a: np.ascontiguousarray(np.asarray(a, dtype=np.float32)
                                         .astype(np.float16))
    x = f32(x).reshape(B, C, N)

    def w_chunked(wT16):  # [C, DK] f16 -> [128, KC*DK] per-partition bytes
        return np.ascontiguousarray(
            wT16.reshape(KC, P, DK).transpose(1, 0, 2).reshape(P, KC * DK))

    def w2_chunked(wT16):  # [C, DK] -> [128, KC*128] doubled [W|W]
        w2 = np.concatenate([wT16, wT16], axis=1)  # [C, 128]
        return np.ascontiguousarray(
            w2.reshape(KC, P, P).transpose(1, 0, 2).reshape(P, KC * P))

    img = np.zeros((P, PKB), np.uint8)
    img[:, OFF_WQ:OFF_WQ + 1024] = w2_chunked(f16(np.asarray(Wq).T)).view(np.uint8)
    img[:, OFF_WK:OFF_WK + 1024] = w2_chunked(f16(np.asarray(Wk).T)).view(np.uint8)
    img[:, OFF_WD:OFF_WD + 512] = w_chunked(f16(np.asarray(Wd).T)).view(np.uint8)
    wub = np.concatenate([f16(np.asarray(Wu).T), f16(bu)[None, :]], axis=0)
    img[0:DK + 1, OFF_WUB:OFF_WUB + 1024] = np.ascontiguousarray(wub).view(np.uint8)
    bq2 = np.concatenate([f32(bq), f32(bq)])
    bk2 = np.concatenate([f32(bk), f32(bk)])
    img[:, OFF_BQ:OFF_BQ + 4] = bq2[:, None].view(np.uint8)
    img[:, OFF_BK:OFF_BK + 4] = bk2[:, None].view(np.uint8)
    img[0:DK, OFF_BD:OFF_BD + 4] = f32(bd)[:, None].view(np.uint8)
    img[0:DK, OFF_GC:OFF_GC + 4] = np.broadcast_to(
        f32(gamma_c)[:, None], (DK, 1)).copy().view(np.uint8)
    img[:, OFF_GS:OFF_GS + 4] = np.broadcast_to(
        f32(gamma_s)[:, None], (P, 1)).copy().view(np.uint8)
    img[:, OFF_BDB:OFF_BDB + 256] = np.broadcast_to(
        f32(bd)[None, :], (P, DK)).copy().view(np.uint8)
    img[:, OFF_BVB:OFF_BVB + 2048] = np.broadcast_to(
        f32(bv)[None, :], (P, C)).copy().view(np.uint8)

    import ml_dtypes
    f8 = lambda a: np.ascontiguousarray(np.asarray(a, dtype=np.float32)
                                        .astype(ml_dtypes.float8_e4m3))
    shared = {
        'wv8': f8(np.asarray(Wv).T),
        'consts': img,
    }
    in_maps = []
    for core in range(NCORES):
        b, h = divmod(core, 2)
        own = slice(h * M, (h + 1) * M)
        other = slice((1 - h) * M, (2 - h) * M)
        xp = np.concatenate([x[b][:, own], x[b][:, other]], axis=1)
        in_maps.append({
            'x': f16(xp),
            'x8': f8(xp),
            'xmT': f16(x[b][:, own].T),
            **shared,
        })
    return in_maps


def assemble_out(results):
    """Stitch the 8 per-core [M, C] outputs back to [B, C, W, H]."""
    full = np.empty((B, C, N), np.float32)
    for core, res in enumerate(results):
        b, h = divmod(core, 2)
        full[b][:, h * M:(h + 1) * M] = res['out'].T.astype(np.float32)
    return full.reshape(B, C, WIDTH, HEIGHT)


def kernel(**inputs):
    nc = _get_compiled()
    in_maps = make_in_maps(**inputs)
    res = bass_utils.run_bass_kernel_spmd(nc, in_maps, core_ids=list(range(NCORES)))
    return assemble_out(res.results)


# revision 9
# speedup vs baseline: 1.0953x; 1.0953x over previous
"""Dual attention (DANet-style spatial + channel attention) on 8 Trainium2
NeuronCores.

Sharding: data-parallel over batch B=4, each batch's output positions split in
half across 2 cores -> 8 identical single-core programs, no collectives. The
host permutes each core's x so its OWN m-half occupies columns [0, M).

Per-core math (x: [512, 4096] f16, m-chunk: 2048 positions):
  spatial:  q=Wq@x[:, :M]+bq; k=Wk@x+bk; E^T[n,m]=k[:,n].q[:,m]
            p_t[n,m] = exp(E - 45)  (bf16, ACT bias)
            S1[m] = sum_n p_t  via tiny PE ones-matmuls (f32 PSUM)
            sbcast = broadcast(1/S1) via 4 tiny PE transposes + outer product
            P~ = e4m3(p_t * sbcast)  -- normalized softmax weights in fp8
            U^T[m,c] = sum_n P~ vT8  via fp8 DoubleRow pairs (2 n-tiles/pass)
            out = gamma_s * U + R    (no 1/Z: P~ is already normalized)
  channel:  pT[n,d]=(Wd@x+bd)^T; e=pT^T@pT; c_attn=softmax(rowmax(e)-e)
            c2=gamma_c*(c_attn@p)[:, :M]+p[:, :M]
            R^T[m,c]=Wu@c2+bu+x[:, :M]^T  (residual DMA'd into r_sb, in-place
            add; bu via appended ones-row)
  out^T[m,c] f16 -> DRAM [M, C], host transposes.

Perf structure (vs the 245us bf16-U version):
  - U matmuls in fp8e4 DoubleRow: the softmax weights are normalized to
    (0, 1] by construction (exact column-sum shift), so they fit e4m3 with
    no range machinery; ~1.6x on the dominant PE cost.
  - the S-accumulation chain moved off the DVE onto tiny PE ones-matmuls
    (one [128,1] matmul per (tile, m-subtile), one PSUM group per chunk).
  - the DVE's new cost is one [128,1024] multiply->fp8 per tile pair
    (issued one pair AHEAD of the consuming U group so the PE never waits).
  - vT stored e4m3 (DR rhs), r_sb f16 (residual DMA'd straight into it),
    output f16 (halves out-DMA).
"""
import sys

sys.path.insert(0, '/opt/trn_rl_repo')

import numpy as np

import concourse.bass as bass
import concourse.tile as tile
from concourse import bacc, bass_utils, mybir
from concourse.masks import make_identity

B, C, WIDTH, HEIGHT = 4, 512, 64, 64
N = WIDTH * HEIGHT      # 4096 spatial positions
DK = 64                 # attention inner dim (and channel-attn dim)
NCORES = 8
M = N // 2              # 2048 output positions per core
P = 128
KC = C // P             # 4 input-channel chunks
NT = N // P             # 32 key-position tiles
NP = NT // 2            # 16 key-tile PAIRS (fp8 DoubleRow granularity)
FREE = 512              # matmul moving free dim (one PSUM bank of fp32)
MCH = M // FREE         # 4 m-chunks per core
MS = FREE // P          # 4 m-subtiles (128 rows) per chunk
MT = M // P             # 16 m-subtiles total

F32 = mybir.dt.float32
F16 = mybir.dt.float16
BF16 = mybir.dt.bfloat16
FP8 = mybir.dt.float8e4
AX = mybir.AxisListType
ALU = mybir.AluOpType
ACTF = mybir.ActivationFunctionType
DR = mybir.MatmulPerfMode.DoubleRow

EXP_BIAS = -45.0        # exp(E + EXP_BIAS): keeps bf16/f32 mid-chain in range

# byte layout of the packed-constants image (per partition)
OFF_WQ, OFF_WK = 0, 1024     # [128, kc, 128] f16: [Wq|Wq], [Wk|Wk] doubled
OFF_WD = 2048                # [128, kc, 64] f16
OFF_BQ, OFF_BK, OFF_BD, OFF_GC = 2560, 2564, 2568, 2572
OFF_GS = 2576                # [128, 1] f32, replicated on all partitions
OFF_P2 = 2592                # consts DMA split: everything below lands first
OFF_WUB = 2592               # [65, 512] f16: rows 0-63 Wu^T, row 64 = bu
OFF_BDB = 3616               # [128, 64] f32, bd row replicated on all partitions
OFF_BVB = 3872               # [128, 512] f32, bv row replicated on all partitions
PKB = 5920


def _build_program(tc, io):
    nc = tc.nc
    x_d, xmT_d, out_d = io['x'], io['xmT'], io['out']

    const_cm = tc.tile_pool(name='const', bufs=1)
    const = const_cm.__enter__()

    # ---- persistent SBUF tensors ----
    pk_sb = const.tile([P, PKB], mybir.dt.uint8)
    nc.sync.dma_start(pk_sb[:, 0:OFF_P2], io['consts'][:, 0:OFF_P2])
    wq_sb = pk_sb[:, OFF_WQ:OFF_WQ + 1024].bitcast(F16).rearrange(
        "p (kc d) -> p kc d", kc=KC)
    wk_sb = pk_sb[:, OFF_WK:OFF_WK + 1024].bitcast(F16).rearrange(
        "p (kc d) -> p kc d", kc=KC)
    wd_sb = pk_sb[:, OFF_WD:OFF_WD + 512].bitcast(F16).rearrange(
        "p (kc d) -> p kc d", kc=KC)
    wub_sb = pk_sb[0:DK + 1, OFF_WUB:OFF_WUB + 1024].bitcast(F16)
    bq_sb = pk_sb[:, OFF_BQ:OFF_BQ + 4].bitcast(F32)
    bk_sb = pk_sb[:, OFF_BK:OFF_BK + 4].bitcast(F32)
    bd_sb = pk_sb[0:DK, OFF_BD:OFF_BD + 4].bitcast(F32)
    gc_sb = pk_sb[0:DK, OFF_GC:OFF_GC + 4].bitcast(F32)
    gs_sb = pk_sb[:, OFF_GS:OFF_GS + 4].bitcast(F32)
    bdb_sb = pk_sb[:, OFF_BDB:OFF_BDB + 256].bitcast(F32)
    bvb_sb = pk_sb[:, OFF_BVB:OFF_BVB + 2048].bitcast(F32)

    ones_colb = const.tile([P, 1], BF16)    # rhs for the tiny S1 matmuls
    nc.vector.memset(ones_colb[:], 1.0)
    ones_rowb = const.tile([1, P], BF16)    # lhsT for the sbcast outer product
    nc.vector.memset(ones_rowb[:], 1.0)
    bias45 = const.tile([P, 1], F32)        # exp bias
    nc.vector.memset(bias45[:], EXP_BIAS)
    ident16 = const.tile([DK, DK], F16)     # for the tiny c_attn transpose
    make_identity(nc, ident16[:])
    identb = const.tile([P, P], BF16)       # for the 1/S1 row transposes
    make_identity(nc, identb[:])

    k_sb = const.tile([P, N], F16)   # keys [d, n], rows 64-127 = copy
    q_sb = const.tile([P, M], F16)   # queries,   rows 64-127 = copy
    pc_sb = const.tile([DK, M], F16)       # channel proj on the m-slice
    c2b_sb = const.tile([DK + 1, M], F16)  # c2 rows 0-63, row 64 = ones
    pT_sb = const.tile([P, NT, DK], F16)   # channel proj transposed [n, nt, d]
    vT8_sb = const.tile([P, NT, C], FP8)   # values transposed e4m3, [n, nt, c]
    catT_sb = const.tile([DK, DK], F16)    # c_attn^T for the c2 matmuls
    r_sb = const.tile([P, MT, C], F16)     # R^T = channel-out + residual

    nc.vector.memset(c2b_sb[DK:DK + 1, :], 1.0)

    wv8_sb = const.tile([P, KC, C], FP8)   # fp8 Wv^T for DR

    out_r = out_d.rearrange("(mt p) c -> p mt c", p=P)

    # ---- pools ----
    # PSUM budget (8 banks): upool 3 + epool 3 + s1 1 + aux 1. U runs in
    # TWO passes per chunk (A: ms 0-1, B: ms 2-3, re-streaming the fp8
    # pairs) so 3 banks suffice; the freed bank goes to epool so E pairs
    # never serialize on the exp releasing a PSUM buffer.
    # During the fused phase upool is not yet entered: epool 3 + s1 1 +
    # auxf 2 (pv/pt rotation) stays within 8 alongside ps0(3, conv scope)
    # ... ps0 closes before epool/auxf enter, as in the conv phase below.
    s1p_cm = tc.tile_pool(name='s1p', bufs=1, space='PSUM')
    s1p = s1p_cm.__enter__()
    epool_cm = tc.tile_pool(name='epool', bufs=3, space='PSUM')
    aux_cm = tc.tile_pool(name='aux', bufs=1, space='PSUM')
    upool_cm = tc.tile_pool(name='upool', bufs=3, space='PSUM')
    upool = epool = aux = None

    ptp_cm = tc.tile_pool(name='pt', bufs=18)    # bf16 exp pairs [P, 2, FREE]
    ptp = ptp_cm.__enter__()
    p8p_cm = tc.tile_pool(name='p8', bufs=20)    # fp8 scaled pairs (a full
    p8p = None                                   # chunk stays live for pass B)
    ssb_cm = tc.tile_pool(name='ssb', bufs=4)    # srh / srow / sbcast
    ssb = ssb_cm.__enter__()
    otp_cm = tc.tile_pool(name='ot', bufs=4)     # f16 epilogue tiles
    otp = otp_cm.__enter__()

    pair_tl = {}   # (mc, pj) -> bf16 exp pair tile
    s1_ps = {}     # mc -> [128, MS] f32 PSUM accumulation tile
    sbc = {}       # mc -> [P, 2, FREE] bf16 broadcast of 1/S1
    u_ps = {}      # mc -> list of MS U PSUM tiles

    def emit_e(mc, nt, epool):
        # nt parity picks the PE row-group: even tiles compute on array rows
        # 0-63, odd on 64-127 (k/q carry identical copies on partitions
        # 64-127), so adjacent E matmuls can overlap in the array.
        msl = slice(mc * FREE, (mc + 1) * FREE)
        nsl = slice(nt * P, (nt + 1) * P)
        h = (nt & 1) * DK
        e_t = epool.tile([P, FREE], F32, tag='et', name=f'et{mc}_{nt}')
        nc.tensor.matmul(e_t[:], lhsT=k_sb[h:h + DK, nsl],
                         rhs=q_sb[h:h + DK, msl],
                         start=True, stop=True, tile_position=(h, 0))
        pj = nt // 2
        if nt % 2 == 0:
            pair = ptp.tile([P, 2, FREE], BF16, tag='p', name=f'p{mc}_{pj}')
            pair_tl[(mc, pj)] = pair
        pair = pair_tl[(mc, pj)]
        nc.scalar.activation(pair[:, nt % 2], e_t[:], ACTF.Exp,
                             bias=bias45[:])

    def emit_s1(mc, nt):
        # S1[m] += column-sums of the exp tile: 4 tiny ones-matmuls into a
        # single per-chunk PSUM accumulation group (one start, one stop).
        if mc not in s1_ps:
            s1_ps[mc] = s1p.tile([P, MS], F32, tag='s1', name=f's1_{mc}')
        pair = pair_tl[(mc, nt // 2)]
        for ms in range(MS):
            nc.tensor.matmul(s1_ps[mc][:, ms:ms + 1],
                             lhsT=pair[:, nt % 2, ms * P:(ms + 1) * P],
                             rhs=ones_colb[:],
                             start=(nt == 0 and ms == 0),
                             stop=(nt == NT - 1 and ms == MS - 1))

    def emit_boundary(mc, aux):
        # After S1(mc) stops: 1/S1 -> bf16 row -> broadcast to [P, 2, FREE].
        srh = ssb.tile([P, MS], BF16, tag='srh', name=f'srh{mc}')
        with nc.allow_low_precision(reason='1/S1 scale: bf16 is plenty'):
            nc.vector.reciprocal(srh[:], s1_ps[mc][:])
        srow = ssb.tile([1, FREE], BF16, tag='srow', name=f'srow{mc}')
        for ms in range(MS):
            ttp = aux.tile([1, P], BF16, tag='aux', name=f'tr{mc}_{ms}')
            nc.tensor.transpose(ttp[:], srh[:, ms:ms + 1], identb[:])
            nc.vector.tensor_copy(srow[0:1, ms * P:(ms + 1) * P], ttp[:])
        sb_ps = aux.tile([P, FREE], F32, tag='aux', name=f'sbp{mc}')
        nc.tensor.matmul(sb_ps[:], lhsT=ones_rowb[:], rhs=srow[:],
                         start=True, stop=True)
        sbcast = ssb.tile([P, 2, FREE], BF16, tag='sbc', name=f'sbc{mc}')
        nc.vector.tensor_copy(sbcast[:, 0], sb_ps[:])
        nc.vector.tensor_copy(sbcast[:, 1], sb_ps[:])
        sbc[mc] = sbcast

    def emit_scale(mc, pj):
        # P~ pair = e4m3(p_t * 1/S1[m]) — one [128, 1024] DVE multiply.
        pair = pair_tl.pop((mc, pj))
        p8 = p8p.tile([P, 2, FREE], FP8, tag='p8', name=f'p8_{mc}_{pj}')
        nc.vector.tensor_mul(p8[:].rearrange("p a b -> p (a b)"),
                             in0=pair[:].rearrange("p a b -> p (a b)"),
                             in1=sbc[mc][:].rearrange("p a b -> p (a b)"))
        return p8

    def emit_u_half(mc, pj, half, p8):
        # 2 fp8 DoubleRow matmuls (two n-tiles per pass) for one ms pair.
        # U runs in two passes over the chunk's fp8 pairs so only 2-3 U
        # PSUM banks are ever live (the third bank buys epool slack).
        for ms in (2 * half, 2 * half + 1):
            nc.tensor.matmul(u_ps[mc][ms][:],
                             lhsT=p8[:, :, ms * P:(ms + 1) * P],
                             rhs=vT8_sb[:, 2 * pj:2 * pj + 2],
                             start=(pj == 0), stop=(pj == NP - 1),
                             perf_mode=DR)

    def emit_combine(mc, ms_list):
        # out = gamma_s * U + R, straight from PSUM (no Z: P~ is normalized).
        for ms in ms_list:
            o2 = otp.tile([P, FREE], F16, tag='o', name=f'o{mc}_{ms}')
            nc.vector.scalar_tensor_tensor(
                out=o2[:], in0=u_ps[mc][ms][:], scalar=gs_sb[:],
                in1=r_sb[:, mc * MS + ms], op0=ALU.mult, op1=ALU.add)
            nc.sync.dma_start(out_r[:, mc * MS + ms], o2[:])

    def channel_step(step, aux):
        # c2 = gamma_c * (c_attn @ p)[:, :M] + pc  (4 matmuls), then
        # R^T[mt] = (c2 | ones)^T @ (Wu^T | bu) + xmT  (16 matmuls, in-place
        # add into r_sb which already holds the residual).
        if step < MCH:
            j = step
            sl = slice(j * FREE, (j + 1) * FREE)
            co_ps = aux.tile([DK, FREE], F32, tag='aux', name=f'co{j}')
            nc.tensor.matmul(co_ps[:], lhsT=catT_sb[:], rhs=pc_sb[:, sl],
                             start=True, stop=True)
            nc.vector.scalar_tensor_tensor(
                out=c2b_sb[0:DK, sl], in0=co_ps[:], scalar=gc_sb[:],
                in1=pc_sb[:, sl], op0=ALU.mult, op1=ALU.add)
        else:
            mt = step - MCH
            rw_ps = aux.tile([P, C], F32, tag='aux', name=f'rw{mt}')
            nc.tensor.matmul(rw_ps[:], lhsT=c2b_sb[:, mt * P:(mt + 1) * P],
                             rhs=wub_sb[:], start=True, stop=True)
            nc.vector.tensor_add(r_sb[:, mt], in0=rw_ps[:], in1=r_sb[:, mt])

    # ================= phase 1 + fused chunk 0 =================
    with tc.tile_pool(name='xp', bufs=1) as xp:
        x_sb = xp.tile([P, KC, N], F16)
        x8_sb = xp.tile([P, KC, N], FP8)
        x_r = x_d.rearrange("(kc p) n -> p kc n", p=P)
        x8_r = io['x8'].rearrange("(kc p) n -> p kc n", p=P)
        for kc in range(KC):   # chunk 0 lands per-kc: conv 0 starts sooner
            nc.sync.dma_start(x_sb[:, kc, 0:FREE], x_r[:, kc, 0:FREE])
        nc.sync.dma_start(pk_sb[:, OFF_P2:PKB], io['consts'][:, OFF_P2:PKB])
        nc.sync.dma_start(x_sb[:, :, FREE:2 * FREE], x_r[:, :, FREE:2 * FREE])
        nc.sync.dma_start(x_sb[:, :, 2 * FREE:4 * FREE],
                          x_r[:, :, 2 * FREE:4 * FREE])
        nc.sync.dma_start(x_sb[:, :, 4 * FREE:6 * FREE],
                          x_r[:, :, 4 * FREE:6 * FREE])
        nc.sync.dma_start(x_sb[:, :, 6 * FREE:N], x_r[:, :, 6 * FREE:N])
        nc.sync.dma_start(wv8_sb[:],
                          io['wv8'].rearrange("(kc p) c -> p kc c", p=P))
        nc.sync.dma_start(x8_sb[:, :, 0:N // 2], x8_r[:, :, 0:N // 2])
        nc.sync.dma_start(x8_sb[:, :, N // 2:N], x8_r[:, :, N // 2:N])
        # residual lands straight in r_sb; channel tail adds in place
        nc.sync.dma_start(r_sb[:],
                          xmT_d.rearrange("(mt p) c -> p mt c", p=P))

        # conv projections, consuming x chunks as they land
        with tc.tile_pool(name='ps0', bufs=3, space='PSUM') as ps0:
            for j in range(8):
                sl = slice(j * FREE, (j + 1) * FREE)
                if j < MCH:
                    pq = ps0.tile([P, FREE], F32, tag='pq', name=f'pq{j}')
                    for kc in range(KC):
                        nc.tensor.matmul(pq[:], lhsT=wq_sb[:, kc],
                                         rhs=x_sb[:, kc, sl],
                                         start=(kc == 0), stop=(kc == KC - 1))
                    nc.scalar.activation(q_sb[:, sl], pq[:], ACTF.Identity,
                                         bias=bq_sb[:])
                    ppc = ps0.tile([DK, FREE], F32, tag='pq', name=f'ppc{j}')
                    for kc in range(KC):
                        nc.tensor.matmul(ppc[:], lhsT=wd_sb[:, kc],
                                         rhs=x_sb[:, kc, sl],
                                         start=(kc == 0), stop=(kc == KC - 1))
                    nc.scalar.activation(pc_sb[:, sl], ppc[:], ACTF.Identity,
                                         bias=bd_sb[:])
                pk = ps0.tile([P, FREE], F32, tag='pq', name=f'pk{j}')
                for kc in range(KC):
                    nc.tensor.matmul(pk[:], lhsT=wk_sb[:, kc],
                                     rhs=x_sb[:, kc, sl],
                                     start=(kc == 0), stop=(kc == KC - 1))
                nc.scalar.activation(k_sb[:, sl], pk[:], ACTF.Identity,
                                     bias=bk_sb[:])
        epool = epool_cm.__enter__()

        # fused chunk-0 loop: produce vT8/pT for tile nt, plus chunk 0's
        # E/exp/S1. No U here (U(0) waits for S1(0) -> runs next period).
        with tc.tile_pool(name='auxf', bufs=2, space='PSUM') as auxf:
            emit_e(0, 0, epool)
            emit_e(0, 1, epool)
            for nt in range(NT):
                nsl = slice(nt * P, (nt + 1) * P)
                pv = auxf.tile([P, C], F32, tag='aux', name=f'pv{nt}')
                for kcp in range(KC // 2):
                    nc.tensor.matmul(
                        pv[:], lhsT=x8_sb[:, 2 * kcp:2 * kcp + 2, nsl],
                        rhs=wv8_sb[:, 2 * kcp:2 * kcp + 2],
                        start=(kcp == 0), stop=(kcp == KC // 2 - 1),
                        perf_mode=DR)
                nc.vector.tensor_add(vT8_sb[:, nt], in0=pv[:], in1=bvb_sb[:])

                pt_ps = auxf.tile([P, DK], F32, tag='aux', name=f'ptp{nt}')
                for kc in range(KC):
                    nc.tensor.matmul(pt_ps[:], lhsT=x_sb[:, kc, nsl],
                                     rhs=wd_sb[:, kc],
                                     start=(kc == 0), stop=(kc == KC - 1))
                nc.vector.tensor_add(pT_sb[:, nt], in0=pt_ps[:], in1=bdb_sb[:])

                if nt % 2 == 1 and nt + 2 < NT:
                    emit_e(0, nt + 1, epool)
                    emit_e(0, nt + 2, epool)
                if nt >= 2:
                    # S1 lags the exp by 2 tiles so the in-order PE queue
                    # never blocks on a pending ACT exp semaphore
                    emit_s1(0, nt - 2)
            emit_s1(0, NT - 2)
            emit_s1(0, NT - 1)

    aux = aux_cm.__enter__()
    upool = upool_cm.__enter__()
    p8p = p8p_cm.__enter__()

    # ================= chunks: U(mc-1) + E/exp/S1(mc) =================
    # Period mc: pass A streams U(mc-1) ms 0-1 interleaved with chunk mc's
    # E/exp/S1; pass B re-streams the fp8 pairs for ms 2-3 while the
    # boundary chain (1/S1 -> sbcast) for chunk mc overlaps the DR stream.
    emit_boundary(0, aux)
    p8_next = emit_scale(0, 0)
    p8_live = {}
    for mc in range(1, MCH + 1):
        last = mc == MCH
        u_ps[mc - 1] = [None] * MS
        for ms in (0, 1):
            u_ps[mc - 1][ms] = upool.tile([P, FREE], F32, tag='u',
                                          name=f'u{mc - 1}_{ms}')
        if not last:
            emit_e(mc, 0, epool)
            emit_e(mc, 1, epool)
        for pj in range(NP):
            nt = 2 * pj
            if not last and pj >= 1:
                # S1 lags its exps by a full pair: no PE-queue stall
                emit_s1(mc, nt - 2)
                emit_s1(mc, nt - 1)
            p8 = p8_next
            p8_live[pj] = p8
            if pj + 1 < NP:
                p8_next = emit_scale(mc - 1, pj + 1)
            emit_u_half(mc - 1, pj, 0, p8)
            if not last and nt + 3 < NT:
                # E pair pj+1 issues after U(pj): exp(pair pj) has drained
                # by then, so the epool buffer rotation never blocks the PE
                emit_e(mc, nt + 2, epool)
                emit_e(mc, nt + 3, epool)
            if mc == 1:
                # channel attention interleave: gram over pT, softmax,
                # c_attn transpose, then c2 / R^T steps two per pair
                if pj < 4:
                    if pj == 0:
                        e_ps = aux.tile([DK, DK], F32, tag='aux',
                                        name='gram')
                    for g in range(8):
                        gi = pj * 8 + g
                        nc.tensor.matmul(e_ps[:], lhsT=pT_sb[:, gi],
                                         rhs=pT_sb[:, gi],
                                         start=(gi == 0),
                                         stop=(gi == NT - 1))
                if pj == 4:
                    # c_attn = softmax(rowmax(e) - e) == exp(rowmin - e)/sum
                    e_sb = const.tile([DK, DK], F32)
                    nc.vector.tensor_copy(e_sb[:], e_ps[:])
                    mn_sb = const.tile([DK, 1], F32)
                    nc.vector.tensor_reduce(mn_sb[:], e_sb[:], axis=AX.X,
                                            op=ALU.min)
                    h_sb = const.tile([DK, DK], F32)
                    nc.scalar.activation(h_sb[:], e_sb[:], ACTF.Exp,
                                         bias=mn_sb[:], scale=-1.0)
                    zc_sb = const.tile([DK, 1], F32)
                    nc.vector.tensor_reduce(zc_sb[:], h_sb[:], axis=AX.X,
                                            op=ALU.add)
                    nc.vector.reciprocal(zc_sb[:], zc_sb[:])
                    cat16_sb = const.tile([DK, DK], F16)
                    nc.vector.tensor_scalar_mul(cat16_sb[:], in0=h_sb[:],
                                                scalar1=zc_sb[:])
                if pj == 5:
                    catT_ps = aux.tile([DK, DK], F16, tag='aux',
                                       name='catT_ps')
                    nc.tensor.transpose(catT_ps[:], cat16_sb[:],
                                        ident16[:])
                    nc.vector.tensor_copy(catT_sb[:], catT_ps[:])
                if 6 <= pj:
                    channel_step(2 * (pj - 6), aux)
                    channel_step(2 * (pj - 6) + 1, aux)
        if not last:
            emit_s1(mc, NT - 2)
            emit_s1(mc, NT - 1)
        emit_combine(mc - 1, (0, 1))   # frees 2 bufs for pass B's rotation
        # ---- pass B: ms 2-3, boundary(mc) overlapped into the stream ----
        for ms in (2, 3):
            u_ps[mc - 1][ms] = upool.tile([P, FREE], F32, tag='u',
                                          name=f'u{mc - 1}_{ms}')
        for pj in range(NP):
            emit_u_half(mc - 1, pj, 1, p8_live[pj])
            if pj == 2 and not last:
                # S1(mc) has stopped by now; the recip/transpose/outer
                # chain runs under the DR stream, then chunk mc's first
                # scale primes so pass A of the next period never waits
                emit_boundary(mc, aux)
                p8_next = emit_scale(mc, 0)
        p8_live.clear()
        emit_combine(mc - 1, (2, 3))

    p8p_cm.__exit__(None, None, None)
    upool_cm.__exit__(None, None, None)
    aux_cm.__exit__(None, None, None)
    epool_cm.__exit__(None, None, None)
    otp_cm.__exit__(None, None, None)
    ssb_cm.__exit__(None, None, None)
    ptp_cm.__exit__(None, None, None)
    s1p_cm.__exit__(None, None, None)
    const_cm.__exit__(None, None, None)


_CACHE = {}


def _get_compiled():
    if 'nc' in _CACHE:
        return _CACHE['nc']
    nc = bacc.Bacc("TRN2", num_devices=NCORES)
    io = {
        'x': nc.dram_tensor('x', [C, N], F16, kind='ExternalInput').ap(),
        'x8': nc.dram_tensor('x8', [C, N], FP8, kind='ExternalInput').ap(),
        'wv8': nc.dram_tensor('wv8', [C, C], FP8, kind='ExternalInput').ap(),
        'xmT': nc.dram_tensor('xmT', [M, C], F16, kind='ExternalInput').ap(),
        'consts': nc.dram_tensor('consts', [P, PKB], mybir.dt.uint8,
                                 kind='ExternalInput').ap(),
        'out': nc.dram_tensor('out', [M, C], F16, kind='ExternalOutput').ap(),
    }
    with tile.TileContext(nc) as tc:
        _build_program(tc, io)
    nc.compile()
    _CACHE['nc'] = nc
    return nc


def make_in_maps(x, Wq, bq, Wk, bk, Wv, bv, gamma_s, Wd, bd, Wu, bu, gamma_c):
    """Build the 8 per-core input dicts from the full problem inputs."""
    f32 = lambda a: np.ascontiguousarray(np.asarray(a, dtype=np.float32))
    f16 = lambda a: np.ascontiguousarray(np.asarray(a, dtype=np.float32)
                                         .astype(np.float16))
    x = f32(x).reshape(B, C, N)

    def w_chunked(wT16):  # [C, DK] f16 -> [128, KC*DK] per-partition bytes
        return np.ascontiguousarray(
            wT16.reshape(KC, P, DK).transpose(1, 0, 2).reshape(P, KC * DK))

    def w2_chunked(wT16):  # [C, DK] -> [128, KC*128] doubled [W|W]
        w2 = np.concatenate([wT16, wT16], axis=1)  # [C, 128]
        return np.ascontiguousarray(
            w2.reshape(KC, P, P).transpose(1, 0, 2).reshape(P, KC * P))

    img = np.zeros((P, PKB), np.uint8)
    img[:, OFF_WQ:OFF_WQ + 1024] = w2_chunked(f16(np.asarray(Wq).T)).view(np.uint8)
    img[:, OFF_WK:OFF_WK + 1024] = w2_chunked(f16(np.asarray(Wk).T)).view(np.uint8)
    img[:, OFF_WD:OFF_WD + 512] = w_chunked(f16(np.asarray(Wd).T)).view(np.uint8)
    wub = np.concatenate([f16(np.asarray(Wu).T), f16(bu)[None, :]], axis=0)
    img[0:DK + 1, OFF_WUB:OFF_WUB + 1024] = np.ascontiguousarray(wub).view(np.uint8)
    bq2 = np.concatenate([f32(bq), f32(bq)])
    bk2 = np.concatenate([f32(bk), f32(bk)])
    img[:, OFF_BQ:OFF_BQ + 4] = bq2[:, None].view(np.uint8)
    img[:, OFF_BK:OFF_BK + 4] = bk2[:, None].view(np.uint8)
    img[0:DK, OFF_BD:OFF_BD + 4] = f32(bd)[:, None].view(np.uint8)
    img[0:DK, OFF_GC:OFF_GC + 4] = np.broadcast_to(
        f32(gamma_c)[:, None], (DK, 1)).copy().view(np.uint8)
    img[:, OFF_GS:OFF_GS + 4] = np.broadcast_to(
        f32(gamma_s)[:, None], (P, 1)).copy().view(np.uint8)
    img[:, OFF_BDB:OFF_BDB + 256] = np.broadcast_to(
        f32(bd)[None, :], (P, DK)).copy().view(np.uint8)
    img[:, OFF_BVB:OFF_BVB + 2048] = np.broadcast_to(
        f32(bv)[None, :], (P, C)).copy().view(np.uint8)

    import ml_dtypes
    f8 = lambda a: np.ascontiguousarray(np.asarray(a, dtype=np.float32)
                                        .astype(ml_dtypes.float8_e4m3))
    shared = {
        'wv8': f8(np.asarray(Wv).T),
        'consts': img,
    }
    in_maps = []
    for core in range(NCORES):
        b, h = divmod(core, 2)
        own = slice(h * M, (h + 1) * M)
        other = slice((1 - h) * M, (2 - h) * M)
        xp = np.concatenate([x[b][:, own], x[b][:, other]], axis=1)
        in_maps.append({
            'x': f16(xp),
            'x8': f8(xp),
            'xmT': f16(x[b][:, own].T),
            **shared,
        })
    return in_maps


def assemble_out(results):
    """Stitch the 8 per-core [M, C] outputs back to [B, C, W, H]."""
    full = np.empty((B, C, N), np.float32)
    for core, res in enumerate(results):
        b, h = divmod(core, 2)
        full[b][:, h * M:(h + 1) * M] = res['out'].T.astype(np.float32)
    return full.reshape(B, C, WIDTH, HEIGHT)


def kernel(**inputs):
    nc = _get_compiled()
    in_maps = make_in_maps(**inputs)
    res = bass_utils.run_bass_kernel_spmd(nc, in_maps, core_ids=list(range(NCORES)))
    return assemble_out(res.results)
